# revision 38
# baseline (speedup 1.0000x reference)
"""CoAttention kernel for Trainium2, data-parallel over batch across 8 NeuronCores.

Per core (one batch element b):
    query = data1[b] @ Wq + bq                      # [2048, 256]
    key   = data2[b] @ Wk + bk                      # [2048, 256]
    attn  = softmax(SCALE * query @ key^T)          # row-constant terms cancel
    out   = attn @ key + query

Device-side strategy (v21, ~76us vs 109us v4 baseline):
  - The host uploads d1^T in bf16 and d2^T/Wk in fp8e4m3, i-interleaved
    p-major so every DMA descriptor is one 4-8KB contiguous run per
    partition.  d1^T is split into eight 256-column q-chunks so the QT
    projection pipelines with the load stream (weight rows are packed in
    the matching i-permutation; contraction order is free).  No device
    casts, no input transposes; ~5.1 MiB/core input traffic.  Load
    order: wq, d1c0-1, bias, wk, d1c2-3, d2t, d1c4-7; 13 warmup matmuls
    keep the PE busy until the first chunk lands (an idle gap drops the
    PE p-state and halves matmul speed for ~3us of re-ramp).
  - softmax(q@(k+bk)^T) drops bias terms constant along k, and
    sum(attn)==1 makes attn@(key+bk) == attn@key + bk, so the key value
    matrix carries NO bias; bq biases the scores path and (bq+bk) the
    residual path from the same QT PSUM.  The softmax denominator is a
    memset 1.0 column appended to the fp8 key values.
  - The K path is entirely fp8: kt (scores stationary) and key2 (ctx
    moving operand) are both DoubleRow matmuls from the fp8 d2T/Wk.
    Scores-path QT evicts split ACT/DVE so exp #0 fires ~1us after the
    fourth d1 chunk lands; exp stream runs ~90% dense on ACT.
  - Residual Q reaches [q, d] layout via xbar DMA transposes on the
    idle mid-kernel DMA engines: no PE, no PSUM, no DVE eviction.
  - scoresT [k, q] orientation lets exp(scoresT) feed the context
    matmul as the stationary operand; scores and context run in fp8e4m3
    DoubleRow at peak PE rate.  The mid-kernel is PE-capacity-bound, so
    fillers (QT nq2/3, K projections, ctx-h0 halves) are hand-slotted
    between scores units at <=1 unit per exp slot.  ctx for the second
    q-half accumulates kp0-6 inside the exp stream and HOLDS its PSUM
    bank (qb 8-11 in ps_a, qb 12-15 paired into the scores banks as the
    final exps release them); the kp7 part continues the same
    accumulation group after the last exp -- no eviction, no re-add.
    Post-exp divides alternate ACT/DVE, residual adds split GPSIMD/DVE.
  - Output is written in 8 chunks of 256 rows as each completes.
"""

import sys

if "/opt/trn_rl_repo" not in sys.path:
    sys.path.insert(0, "/opt/trn_rl_repo")

from contextlib import ExitStack

import ml_dtypes
import numpy as np

import concourse.bass as bass  # noqa: F401
import concourse.mybir as mybir
import concourse.tile as tile
from concourse import bacc
from concourse.bass_utils import run_bass_kernel_spmd

B, LQ, LK, DIN, D = 8, 2048, 2048, 1024, 256
N_CORES = 8
SCALE = float(1.0 / np.sqrt(1024.0).astype(np.float32))

BF16 = mybir.dt.bfloat16
FP8 = mybir.dt.float8e4
F32 = mybir.dt.float32
AF = mybir.ActivationFunctionType
PM_DR = mybir.MatmulPerfMode.DoubleRow
ADD = mybir.AluOpType.add
MULT = mybir.AluOpType.mult

QB = 16           # q blocks of 128
KB = 16           # k blocks of 128
J1 = 8            # d1 i-interleave factor (1024 = 128 * 8)
J2 = 2            # d2 i-interleave factor (256 = 128 * 2)
KP = KB // 2      # 8 fp8 DoubleRow k-pairs
KP_A = 7          # h1 ctx kp-split: A = kp0-6 inside exp stream, B = kp7 after


def _build():
    nc = bacc.Bacc("TRN2", target_bir_lowering=False, debug=False)
    d1t = nc.dram_tensor("d1t", [8, 128, J1, 256], BF16, kind="ExternalInput").ap()
    d2t = nc.dram_tensor("d2t", [128, J2, LK], FP8, kind="ExternalInput").ap()
    wq_d = nc.dram_tensor("wq", [128, 2048], BF16, kind="ExternalInput").ap()
    wk_d = nc.dram_tensor("wk", [128, 512], FP8, kind="ExternalInput").ap()
    bias = nc.dram_tensor("bias", [128, 4], F32, kind="ExternalInput").ap()
    out = nc.dram_tensor("out", [LQ, D], F32, kind="ExternalOutput").ap()

    with tile.TileContext(nc) as tc, ExitStack() as ctx:
        const = ctx.enter_context(tc.tile_pool(name="const", bufs=1))
        big = ctx.enter_context(tc.tile_pool(name="big", bufs=1))
        stage = ctx.enter_context(tc.tile_pool(name="stage", bufs=3))
        small = ctx.enter_context(tc.tile_pool(name="small", bufs=4))
        ps_a = ctx.enter_context(tc.tile_pool(name="ps_a", bufs=4, space="PSUM"))
        ps_sc = ctx.enter_context(tc.tile_pool(name="ps_sc", bufs=2, space="PSUM"))

        # ---------------- constants / small state ---------------------------
        warm_src = const.tile([128, 512], BF16, tag="warm_src")
        nc.gpsimd.memset(warm_src[:], 0.0)
        dummy = const.tile([128, 1], F32, tag="dummy")
        # force the exp ACT table load at kernel start (otherwise it stalls
        # the first real exp by ~1.3us mid-stream)
        nc.scalar.activation(dummy[:], warm_src[:, 0:1], AF.Exp)

        key2 = [
            big.tile([128, 2, D + 1], FP8, tag=f"key2_{kp}", name=f"key2_{kp}")
            for kp in range(KP)
        ]
        for kp in range(KP):
            nc.gpsimd.memset(key2[kp][:, :, D:D + 1], 1.0)

        # ---------------- loads ---------------------------------------------
        wq_sb = const.tile([128, 2048], BF16, tag="wq_sb")
        wk_sb = const.tile([128, 512], FP8, tag="wk_sb")
        bias_sb = const.tile([128, 4], F32, tag="bias_sb")
        d2T = big.tile([128, J2, LK], FP8, tag="d2T")
        d1T = [big.tile([128, J1, 256], BF16, tag=f"d1T{n}", name=f"d1T{n}")
               for n in range(8)]

        nc.sync.dma_start(out=wq_sb[:], in_=wq_d)
        for n in range(2):
            nc.sync.dma_start(out=d1T[n][:], in_=d1t[n])
        nc.sync.dma_start(out=bias_sb[:], in_=bias)
        nc.sync.dma_start(out=wk_sb[:], in_=wk_d)
        for n in range(2, 4):
            nc.sync.dma_start(out=d1T[n][:], in_=d1t[n])
        nc.sync.dma_start(out=d2T[:], in_=d2t)
        for n in range(4, 8):
            nc.sync.dma_start(out=d1T[n][:], in_=d1t[n])

        # weight slices in the same i-permutation as the activations
        wqs = [wq_sb[:, j * D:(j + 1) * D] for j in range(J1)]
        wks = [wk_sb[:, j * D:(j + 1) * D] for j in range(J2)]
        bq_col = bias_sb[:, 0:2]
        bqk_col = bias_sb[:, 2:4]

        # ---------------- PE p-state warmup (also bridges the d2T wait) -----
        for w in range(13):
            pw = ps_a.tile([128, 512], F32, tag="ps_a", name=f"warm{w}")
            nc.tensor.matmul(pw[:], lhsT=warm_src[:, :128], rhs=warm_src[:],
                             start=True, stop=True)

        # ---------------- K^T fp8 DoubleRow layout [128, 2, k] --------------
        kt_sb = big.tile([128, 2, LK], FP8, tag="kt_sb")

        wk2 = wk_sb[:].rearrange("p (j d) -> p j d", j=J2)

        def kt_unit(dc, nk, on_act):
            ps = ps_a.tile([128, 512], F32, tag="ps_a")
            nc.tensor.matmul(
                ps[:],
                lhsT=wk2[:, :, dc * 128:(dc + 1) * 128],
                rhs=d2T[:, :, nk * 512:(nk + 1) * 512],
                perf_mode=PM_DR,
                start=True,
                stop=True,
            )
            o = kt_sb[:, dc, nk * 512:(nk + 1) * 512]
            if on_act:
                nc.scalar.copy(o, ps[:])
            else:
                nc.vector.tensor_copy(o, ps[:])

        # ---------------- key values via fp8 DoubleRow matmuls ---------------
        def key_tr(kp):
            ps = ps_a.tile([128, 512], F32, tag="ps_a")
            for s in range(2):
                kb = 2 * kp + s
                nc.tensor.matmul(
                    ps[:, s * D:(s + 1) * D],
                    lhsT=d2T[:, :, kb * 128:(kb + 1) * 128],
                    rhs=wk2,
                    perf_mode=PM_DR,
                    start=True,
                    stop=True,
                )
            nc.vector.tensor_copy(
                key2[kp][:, :, :D],
                ps[:].rearrange("p (s d) -> p s d", s=2),
            )

        # ---------------- QT projection ------------------------------------
        qt_sb = big.tile([128, 2, LQ], FP8, tag="qt_sb")
        qtbf = big.tile([128, 2, LQ], BF16, tag="qtbf")

        def qt_bias_sc(ps, dc, nq, on_act):
            o = qt_sb[:, dc, nq * 512:(nq + 1) * 512]
            if on_act:
                nc.scalar.activation(o, ps[:], AF.Identity,
                                     bias=bq_col[:, dc:dc + 1])
            else:
                nc.vector.tensor_scalar(o, ps[:], bq_col[:, dc:dc + 1], None, ADD)

        def qt_bias_rs(ps, dc, nq, on_act):
            o = qtbf[:, dc, nq * 512:(nq + 1) * 512]
            if on_act:
                nc.scalar.activation(o, ps[:], AF.Identity,
                                     bias=bqk_col[:, dc:dc + 1])
            else:
                nc.vector.tensor_scalar(o, ps[:], bqk_col[:, dc:dc + 1], None, ADD)

        qt_ps = {}

        def qt_half(dc, nq, h):
            # one 8-chain over q-chunk c = 2*nq + h into half of the psum tile
            if h == 0:
                qt_ps[(dc, nq)] = ps_a.tile([128, 512], F32, tag="ps_a",
                                            name=f"qtps_{dc}_{nq}")
            ps = qt_ps[(dc, nq)]
            c = 2 * nq + h
            for j in range(J1):
                nc.tensor.matmul(
                    ps[:, h * 256:(h + 1) * 256],
                    lhsT=wqs[j][:, dc * 128:(dc + 1) * 128],
                    rhs=d1T[c][:, j, :],
                    start=(j == 0),
                    stop=(j == J1 - 1),
                )

        def qt_evict(dc, nq, sc_act, rs_act):
            ps = qt_ps[(dc, nq)]
            qt_bias_sc(ps, dc, nq, sc_act)
            qt_bias_rs(ps, dc, nq, rs_act)

        def qt_unit(dc, nq, sc_act, rs_act):
            qt_half(dc, nq, 0)
            qt_half(dc, nq, 1)
            qt_evict(dc, nq, sc_act, rs_act)

        # ---------------- residual Q via xbar DMA transpose ------------------
        # qres3[qg][q_low, j, dc, c] = Q[qg*512 + j*128 + q_low, dc*128 + c]
        qres3 = [big.tile([128, 4, 2, 128], BF16, tag=f"qres{qg}",
                          name=f"qres{qg}")
                 for qg in range(4)]

        def qres_xbar(qg, dc):
            nc.sync.dma_start_transpose(
                out=qres3[qg][:, :, dc, :],
                in_=qtbf[:, dc, qg * 512:(qg + 1) * 512],
            )

        # ---------------- scores + exp --------------------------------------
        expT = [
            [big.tile([128, 2, 1024], FP8, tag=f"expT{kp}_{nh}",
                      name=f"expT{kp}_{nh}")
             for nh in range(2)]
            for kp in range(KP)
        ]

        def scores_unit(km, nh):
            ps = ps_sc.tile([128, 1024], F32, tag="ps_sc")
            for half in range(2):
                nq = nh * 2 + half
                nc.tensor.matmul(
                    ps[:, half * 512:(half + 1) * 512],
                    lhsT=kt_sb[:, :, km * 128:(km + 1) * 128],
                    rhs=qt_sb[:, :, nq * 512:(nq + 1) * 512],
                    perf_mode=PM_DR,
                    start=True,
                    stop=True,
                )
            nc.scalar.activation(
                expT[km // 2][nh][:, km % 2, :], ps[:], AF.Exp, scale=SCALE
            )

        sc_ps = {}

        def scores_half(km, nh, half):
            # nq-granular: the first exps fire off the nq0 evict alone,
            # ~2us before the nq1 chain closes
            if half == 0:
                sc_ps[km] = ps_sc.tile([128, 1024], F32, tag="ps_sc",
                                       name=f"scps{km}")
            ps = sc_ps[km]
            nq = nh * 2 + half
            nc.tensor.matmul(
                ps[:, half * 512:(half + 1) * 512],
                lhsT=kt_sb[:, :, km * 128:(km + 1) * 128],
                rhs=qt_sb[:, :, nq * 512:(nq + 1) * 512],
                perf_mode=PM_DR,
                start=True,
                stop=True,
            )
            nc.scalar.activation(
                expT[km // 2][nh][:, km % 2, half * 512:(half + 1) * 512],
                ps[:, half * 512:(half + 1) * 512], AF.Exp, scale=SCALE
            )

        # ---------------- context + residual + out DMA ----------------------
        out_c = [stage.tile([128, 2 * D], F32, tag="outc", name=f"outc{c}")
                 for c in range(QB // 2)]

        def ctx_mm(pc, qb, kp, start, stop):
            h, qq = qb // 8, qb % 8
            nc.tensor.matmul(
                pc,
                lhsT=expT[kp][h][:, :, qq * 128:(qq + 1) * 128],
                rhs=key2[kp][:],
                perf_mode=PM_DR,
                start=start,
                stop=stop,
            )

        def ctx_finish(pc, qb, div_act=False, add_dve=False):
            rc = small.tile([128, 1], F32, tag="recip")
            nc.vector.reciprocal(rc[:], pc[:, D:D + 1])
            c = qb // 2
            osl = out_c[c][:, (qb % 2) * D:(qb % 2 + 1) * D]
            if div_act:
                nc.scalar.activation(osl, pc[:, :D], AF.Identity, scale=rc[:])
            else:
                nc.vector.tensor_scalar(osl, pc[:, :D], rc[:], None, MULT)
            qg, j = qb // 4, qb % 4
            o2 = osl.rearrange("p (a b) -> p a b", a=2)
            if add_dve:
                nc.vector.tensor_tensor(o2, o2, qres3[qg][:, j, :, :], ADD)
            else:
                nc.gpsimd.tensor_add(o2, o2, qres3[qg][:, j, :, :])
            if qb % 2 == 1:
                nc.sync.dma_start(
                    out=out[c * 256:(c + 1) * 256, :].rearrange(
                        "(t p) d -> p t d", p=128),
                    in_=out_c[c][:].rearrange("p (t d) -> p t d", d=D),
                )

        ctx_pc = {}

        def ctx_h0_a(qb):
            pc_full = ps_a.tile([128, 512], F32, tag="ps_a",
                                name=f"ctxpc{qb}")
            ctx_pc[qb] = pc_full[:, :D + 1]
            for kp in range(4):
                ctx_mm(ctx_pc[qb], qb, kp, kp == 0, False)

        def ctx_h0_b(qb):
            pc = ctx_pc[qb]
            for kp in range(4, KP):
                ctx_mm(pc, qb, kp, False, kp == KP - 1)
            ctx_finish(pc, qb)

        def ctx_h1_A(qb):
            # kp0-6 accumulate and the bank stays LIVE; the kp7 B part
            # continues the same accumulation group after the last exp.
            # qb 8-11 use the ps_a banks; qb 12-15 pair up in the scores
            # PSUM tiles as exps 14/15 release them.
            if qb < 12:
                pc_full = ps_a.tile([128, 512], F32, tag="ps_a",
                                    name=f"ctxh1_{qb}")
                ctx_pc[qb] = pc_full[:, :D + 1]
            else:
                pair = (qb - 12) // 2
                if qb % 2 == 0:
                    ctx_pc[f"p{pair}"] = ps_sc.tile(
                        [128, 1024], F32, tag="ps_sc", name=f"ctxsc{pair}")
                off = (qb % 2) * 512
                ctx_pc[qb] = ctx_pc[f"p{pair}"][:, off:off + D + 1]
            for kp in range(KP_A):
                ctx_mm(ctx_pc[qb], qb, kp, kp == 0, False)

        def ctx_h1_B(qb):
            pc = ctx_pc[qb]
            for kp in range(KP_A, KP):
                ctx_mm(pc, qb, kp, False, kp == KP - 1)
            ctx_finish(pc, qb, div_act=(qb % 2 == 0), add_dve=(qb % 4 >= 2))


        # ================= emission schedule ================================
        def units(fn, idxs):
            return [lambda i=i: fn(*i) if isinstance(i, tuple) else fn(i)
                    for i in idxs]

        def interleave(a, b, ratio):
            a = list(a)
            b = list(b)
            ia = ib = 0
            credit = 0.0
            while ia < len(a) or ib < len(b):
                if ia < len(a):
                    a[ia]()
                    ia += 1
                credit += ratio
                while credit >= 1.0 and ib < len(b):
                    b[ib]()
                    ib += 1
                    credit -= 1.0
            while ib < len(b):
                b[ib]()
                ib += 1

        # --- phase 1: QT nq0/nq1 chunk-paced as the first d1 chunks land
        #     (d1 loads first; no PE gap so the p-state stays hot), then
        #     KT nk0 the moment d2T lands.  Scores-path evicts split
        #     ACT (dc0) / DVE (dc1). ---
        for h in range(2):
            qt_half(0, 0, h)
            qt_half(1, 0, h)
        # all QT evicts on DVE: the ACT queue stays clear for the kt copy
        # and the exp stream (evicts there would sit on exp0's critical path)
        qt_evict(0, 0, sc_act=False, rs_act=False)
        qt_evict(1, 0, sc_act=False, rs_act=False)
        for h in range(2):
            qt_half(0, 1, h)
            qt_half(1, 1, h)
        for dc in range(2):
            kt_unit(dc, 0, on_act=(dc == 0))
        # first two scores units nq-split: their nq0 exps run while the
        # nq1 evicts finish
        scores_half(0, 0, 0)
        scores_half(1, 0, 0)
        qt_evict(0, 1, sc_act=False, rs_act=False)
        qt_evict(1, 1, sc_act=False, rs_act=False)
        scores_half(0, 0, 1)
        scores_half(1, 0, 1)
        for qg in range(2):
            for dc in range(2):
                qres_xbar(qg, dc)

        # --- phase 2: scores-h0 with time-budgeted fillers (<=1 big or 2
        #     small filler units between consecutive scores units) ---
        def qres_late(qg):
            for dc in range(2):
                qres_xbar(qg, dc)

        def qt23h_ev(dc, nq):
            qt_half(dc, nq, 1)
            qt_evict(dc, nq, sc_act=False, rs_act=False)

        slot_fill = {
            2: [lambda: kt_unit(0, 1, False), lambda: kt_unit(1, 1, False)],
            3: [lambda: key_tr(0)],
            4: [lambda: qt_half(0, 2, 0)],
            5: [lambda: qt23h_ev(0, 2)],
            6: [lambda: kt_unit(0, 2, False), lambda: kt_unit(1, 2, False)],
            7: [lambda: qt_half(1, 2, 0)],
            8: [lambda: qt23h_ev(1, 2)],
            9: [lambda: key_tr(1)],
            10: [lambda: kt_unit(0, 3, False), lambda: kt_unit(1, 3, False)],
            11: [lambda: qt_half(0, 3, 0)],
            12: [lambda: qt23h_ev(0, 3)],
            13: [lambda: qt_half(1, 3, 0)],
            14: [lambda: qt23h_ev(1, 3), lambda: qres_late(2)],
            15: [lambda: key_tr(2), lambda: qres_late(3)],
        }
        for km in range(2, KB):
            scores_unit(km, 0)
            for f in slot_fill.get(km, []):
                f()

        # --- phase 3: scores-h1 with ctx-h0 halves (one ~0.5us half per
        #     slot keeps the scores matmuls ahead of the exp stream) ---
        sc_h1 = units(scores_unit, [(km, 1) for km in range(KB)])
        p3_fill = {0: [lambda: key_tr(3), lambda: key_tr(4)],
                   1: [lambda: key_tr(5), lambda: key_tr(6)],
                   2: [lambda: key_tr(7)]}
        # strictly one ~0.5us ctx half per exp slot; the last unit spills
        # into phase 4 where its finish overlaps the A-part matmuls
        for qb in range(7):
            p3_fill.setdefault(2 + 2 * qb, []).append(
                lambda q=qb: ctx_h0_a(q))
            p3_fill.setdefault(3 + 2 * qb, []).append(
                lambda q=qb: ctx_h0_b(q))
        for i, u in enumerate(sc_h1):
            u()
            for f in p3_fill.get(i, []):
                f()

        # --- phase 4: ctx-h1 A parts (need exps km0-13), then the kp7
        #     B parts + finishes after the last exp ---
        ctx_h0_a(7)
        ctx_h0_b(7)
        # 4 A-parts fill the ps_a banks; each B finish frees a bank for
        # the next A.  Only the B matmuls (1 kp each) wait on the final exp.
        for qb in range(8, 12):
            ctx_h1_A(qb)
        ctx_h1_A(12)
        ctx_h1_A(13)
        for qb in range(8, 12):
            ctx_h1_B(qb)
        ctx_h1_A(14)
        ctx_h1_A(15)
        for qb in range(12, 16):
            ctx_h1_B(qb)

    nc.compile()
    return nc


_NC = None
_last_in_maps = None


def make_host_inputs(data1_b, data2_b, Wq, bq, Wk, bk):
    """Pack one batch element's inputs into the device layout (bf16 + f32).

    d1t[n, p, j, q'] = data1[n*512 + q', 8p + j]   (q-chunked, i p-major)
    d2t[p, j, k]     = data2[k, 2p + j]
    wq[p, j*256+d]   = Wq[8p + j, d]; wk[p, j*256+d] = Wk[2p + j, d]
    """
    bf = ml_dtypes.bfloat16
    a1 = np.asarray(data1_b, np.float32).astype(bf)      # [2048, 1024]
    d1t = np.ascontiguousarray(
        a1.reshape(8, 256, 128, J1).transpose(0, 2, 3, 1))
    f8 = ml_dtypes.float8_e4m3
    a2 = np.asarray(data2_b, np.float32).astype(f8)      # [2048, 256]
    d2t = np.ascontiguousarray(
        a2.reshape(LK, 128, J2).transpose(1, 2, 0))
    Wq = np.asarray(Wq, dtype=np.float32)
    Wk = np.asarray(Wk, dtype=np.float32)
    bq = np.asarray(bq, dtype=np.float32)
    bk = np.asarray(bk, dtype=np.float32)
    wq = np.ascontiguousarray(
        Wq.astype(bf).reshape(128, J1, D).reshape(128, J1 * D))
    wk = np.ascontiguousarray(
        Wk.astype(f8).reshape(128, J2, D).reshape(128, J2 * D))
    bias = np.empty((128, 4), np.float32)
    bqk = bq + bk
    for c in range(2):
        bias[:, c] = bq[c * 128:(c + 1) * 128]
        bias[:, 2 + c] = bqk[c * 128:(c + 1) * 128]
    return {"d1t": d1t, "d2t": d2t, "wq": wq, "wk": wk, "bias": bias}


def _get_nc():
    global _NC
    if _NC is None:
        _NC = _build()
    return _NC


def kernel(data1, data2, Wq, bq, Wk, bk):
    global _last_in_maps
    data1 = np.asarray(data1, dtype=np.float32)
    data2 = np.asarray(data2, dtype=np.float32)

    nc = _get_nc()
    shared = None
    in_maps = []
    for b in range(B):
        m = make_host_inputs(data1[b], data2[b], Wq, bq, Wk, bk)
        if shared is None:
            shared = {k: m[k] for k in ("wq", "wk", "bias")}
        m.update(shared)
        in_maps.append(m)
    _last_in_maps = in_maps
    res = run_bass_kernel_spmd(nc, in_maps, core_ids=list(range(N_CORES)))
    return np.stack([res.results[i]["out"] for i in range(B)], axis=0)
